# revision 9
# baseline (speedup 1.0000x reference)
"""Trainium2 Bass kernel for BaseViTSelfAttention (cross/self attention, 16 heads).

Computation (per batch element b):
    q = hidden @ Wq.T            [1024, 1024]
    ctx = concat(hidden, context)  [1280, 1024]
    k = ctx @ Wk.T; v = ctx @ Wv.T
    out = softmax(q_h @ k_h.T / 8) @ v_h   per 64-dim head, reassembled

Sharding: batch-parallel, one batch element per NeuronCore (8 cores).
Host-side prep (numpy, layout only): transpose weights to [di, do] and build
ctxT = concat(hidden, context).transpose -> [D, NK] per batch so the
contraction dim lands on SBUF partitions.

All matmuls run in float32r (fp32 storage read at FP22 by the PE): full PE
rate at out-free-dim >= 256, rel err ~1e-4.

Biases are all-zero for this problem spec and are ignored.
"""
import numpy as np

import concourse.bass as bass
import concourse.mybir as mybir
import concourse.tile as tile
from concourse import bacc
from concourse.bass import ds, ts
from concourse.bass_utils import run_bass_kernel_spmd
from concourse.masks import make_identity

N_CORES = 8
P = 128
D = 1024          # model dim
NQ = 1024         # query length (hidden)
NK = 1280         # key/value length (hidden + context)
H = 16            # heads
DH = 64           # head dim
DT = D // P       # 8 contraction tiles
NKT = NK // P     # 10 nk tiles
SCALE = 1.0 / 8.0  # 1/sqrt(DH)
F32 = mybir.dt.float32
F32R = mybir.dt.float32r


def emit(nc, tc, ctx_d, wq_d, wk_d, wv_d, out_d, repeat=1):
    with tc.tile_pool(name="persist", bufs=1) as persist:
        _emit_body(nc, tc, persist, ctx_d, wq_d, wk_d, wv_d, out_d, repeat)


def _emit_body(nc, tc, persist, ctx_d, wq_d, wk_d, wv_d, out_d, repeat):
    ident = persist.tile([P, P], F32)
    make_identity(nc, ident[:])
    ones_d = nc.inline_tensor(np.ones((P, NKT * H), dtype=np.float32), name="ones")

    for _ in range(repeat):
        qT = persist.tile([P, DT, NQ], F32R, tag="qT")    # [do%128, do//128, nq]
        kT = persist.tile([P, DT, NK], F32R, tag="kT")    # [do%128, do//128, nk]
        v = persist.tile([P, NKT, H, DH + 1], F32R, tag="v")  # natural v + ones col
        nc.sync.dma_start(
            v[:, :, :, DH:DH + 1],
            ones_d[:, :].rearrange("p (t h) -> p t h", t=NKT)[:, :, :, None].bitcast(F32R),
        )

        # ---------------- phase 1: projections ----------------
        with (
            tc.tile_pool(name="p1", bufs=1) as p1,
            tc.tile_pool(name="wp", bufs=8) as wp,
            tc.tile_pool(name="psp", bufs=4, space="PSUM") as psp,
        ):
            ctxT = p1.tile([P, DT, NK], F32R, tag="ctxT")
            for t in range(DT):
                nc.sync.dma_start(
                    ctxT[:, t, :], ctx_d[ts(t, P), :].bitcast(F32R)
                )

            def load_w(w_d):
                tiles = []
                for t in range(DT):
                    wt = wp.tile([P, D], F32R, tag="w")
                    nc.sync.dma_start(wt[:], w_d[ts(t, P), :].bitcast(F32R))
                    tiles.append(wt)
                return tiles

            # Q: qT[do, nq] = sum_di WqT[di, do] * ctxT[di, nq]
            wq = load_w(wq_d)
            for dt in range(DT):
                for c in range(2):
                    ps = psp.tile([P, 512], F32, tag="ps")
                    for di in range(DT):
                        nc.tensor.matmul(
                            ps[:],
                            wq[di][:, ts(dt, P)],
                            ctxT[:, di, ds(c * 512, 512)],
                            start=(di == 0),
                            stop=(di == DT - 1),
                        )
                    nc.vector.tensor_copy(qT[:, dt, ds(c * 512, 512)], ps[:])

            # K: kT[do, nk], nk chunks of (512, 512, 256)
            wk = load_w(wk_d)
            for dt in range(DT):
                for (c0, w) in ((0, 512), (512, 512), (1024, 256)):
                    ps = psp.tile([P, 512], F32, tag="ps")
                    for di in range(DT):
                        nc.tensor.matmul(
                            ps[:, :w],
                            wk[di][:, ts(dt, P)],
                            ctxT[:, di, ds(c0, w)],
                            start=(di == 0),
                            stop=(di == DT - 1),
                        )
                    nc.vector.tensor_copy(kT[:, dt, ds(c0, w)], ps[:, :w])

            # V natural: v[nk, do] = sum_di ctxT[di, nk] * WvT[di, do]
            wv = load_w(wv_d)
            for m in range(NKT):
                for g in range(2):
                    ps = psp.tile([P, 512], F32, tag="ps")
                    for di in range(DT):
                        nc.tensor.matmul(
                            ps[:],
                            ctxT[:, di, ts(m, P)],
                            wv[di][:, ds(g * 512, 512)],
                            start=(di == 0),
                            stop=(di == DT - 1),
                        )
                    nc.vector.tensor_copy(
                        v[:, m, ds(g * 8, 8), 0:DH],
                        ps[:].rearrange("p (h d) -> p h d", h=8),
                    )

        # ---------------- phase 2: attention ----------------
        NQC = 256                # nq chunk for scores/out matmuls
        NCH = NQ // NQC          # 4 chunks
        GROUPS = ((0, 4), (4, 4), (8, 2))   # nk-tile groups per psum tile
        with (
            tc.tile_pool(name="p2", bufs=4) as p2,
            tc.tile_pool(name="stg", bufs=2) as stg,
            tc.tile_pool(name="outp", bufs=4) as outp,
            tc.tile_pool(name="pss", bufs=2, space="PSUM") as pss,
            tc.tile_pool(name="pso", bufs=2, space="PSUM") as pso,
            tc.tile_pool(name="pst", bufs=2, space="PSUM") as pst,
        ):
            for c in range(NCH):
                out_tiles = [
                    outp.tile([P, D], F32, tag="out", name=f"out_{c}_{j}")
                    for j in range(NQC // P)
                ]
                for hp in range(H // 2):
                    pair = (2 * hp, 2 * hp + 1)
                    et = {
                        h: p2.tile([P, NKT, NQC], F32R, tag="expT",
                                   name=f"expT_{h}")
                        for h in pair
                    }
                    # scoresT[nk, nq] = kT_h slice (stationary) x qT_h slice;
                    # the head pair sits at partition offsets 0/64 and is
                    # emitted interleaved -> concurrent row-tiled matmuls.
                    # Several nk-tiles share one multi-bank psum tile so exp
                    # runs as few big ACT instructions.
                    for g0, gl in GROUPS:
                        pp = {
                            h: pss.tile([P, 4, NQC], F32, tag="pss",
                                        name=f"pss_{h}")
                            for h in pair
                        }
                        for tt in range(gl):
                            for h in pair:
                                o = 64 * (h % 2)
                                nc.tensor.matmul(
                                    pp[h][:, tt, :],
                                    kT[o:o + DH, h // 2, ts(g0 + tt, P)],
                                    qT[o:o + DH, h // 2, ds(c * NQC, NQC)],
                                    start=True,
                                    stop=True,
                                )
                        for h in pair:
                            nc.scalar.activation(
                                et[h][:, ds(g0, gl), :], pp[h][:, :gl, :],
                                mybir.ActivationFunctionType.Exp,
                                scale=SCALE,
                            )
                    # outT_aug[65, nq] = sum_nk v_aug[nk, 65] * expT[nk, nq]
                    for h in pair:
                        po = pso.tile([DH + 1, NQC], F32, tag="pso")
                        for t in range(NKT):
                            nc.tensor.matmul(
                                po[:],
                                v[:, t, h, :],
                                et[h][:, t, :],
                                start=(t == 0),
                                stop=(t == NKT - 1),
                            )
                        st = stg.tile([DH + 1, NQC], F32, tag="stage")
                        nc.vector.tensor_copy(st[:], po[:])
                        for j in range(NQC // P):
                            pt = pst.tile([P, DH + 1], F32, tag="pst")
                            nc.tensor.transpose(
                                pt[:], st[:, ts(j, P)], ident[:DH + 1, :DH + 1]
                            )
                            rc = stg.tile([P, 1], F32, tag="recip")
                            nc.vector.reciprocal(rc[:], pt[:, DH:DH + 1])
                            nc.vector.tensor_scalar_mul(
                                out_tiles[j][:, ds(h * DH, DH)],
                                pt[:, 0:DH],
                                rc[:],
                            )
                for j in range(NQC // P):
                    nt = c * (NQC // P) + j
                    nc.sync.dma_start(out_d[ts(nt, P), :], out_tiles[j][:])


_CACHE = {}


def build(repeat=1):
    key = repeat
    if key in _CACHE:
        return _CACHE[key]
    nc = bacc.Bacc("TRN2", target_bir_lowering=False, debug=False,
                   num_devices=N_CORES)
    ctx_d = nc.dram_tensor("ctxT", [D, NK], F32, kind="ExternalInput")
    wq_d = nc.dram_tensor("wqT", [D, D], F32, kind="ExternalInput")
    wk_d = nc.dram_tensor("wkT", [D, D], F32, kind="ExternalInput")
    wv_d = nc.dram_tensor("wvT", [D, D], F32, kind="ExternalInput")
    out_d = nc.dram_tensor("out", [NQ, D], F32, kind="ExternalOutput")
    with tile.TileContext(nc) as tc:
        emit(nc, tc, ctx_d, wq_d, wk_d, wv_d, out_d, repeat=repeat)
    nc.compile()
    _CACHE[key] = (nc, ctx_d, wq_d, wk_d, wv_d, out_d)
    return _CACHE[key]


def make_in_maps(hidden_states, context_states, Wq, Wk, Wv):
    ctxT = np.ascontiguousarray(
        np.concatenate([hidden_states, context_states], axis=1).transpose(0, 2, 1)
    ).astype(np.float32)
    wqT = np.ascontiguousarray(np.asarray(Wq, dtype=np.float32).T)
    wkT = np.ascontiguousarray(np.asarray(Wk, dtype=np.float32).T)
    wvT = np.ascontiguousarray(np.asarray(Wv, dtype=np.float32).T)
    return [
        {"ctxT": ctxT[b], "wqT": wqT, "wkT": wkT, "wvT": wvT}
        for b in range(N_CORES)
    ]


def kernel(hidden_states, context_states, Wq, bq, Wk, bk, Wv, bv):
    # bq/bk/bv are zeros per the problem spec; not applied.
    nc = build(repeat=1)[0]
    in_maps = make_in_maps(hidden_states, context_states, Wq, Wk, Wv)
    res = run_bass_kernel_spmd(nc, in_maps, core_ids=list(range(N_CORES)))
    return np.stack([res.results[b]["out"] for b in range(N_CORES)], axis=0)


# revision 10
# speedup vs baseline: 4.4427x; 4.4427x over previous
"""Trainium2 Bass kernel for BaseViTSelfAttention (cross/self attention, 16 heads).

Computation (per batch element b):
    q = hidden @ Wq.T            [1024, 1024]
    ctx = concat(hidden, context)  [1280, 1024]
    k = ctx @ Wk.T; v = ctx @ Wv.T
    out = softmax(q_h @ k_h.T / 8) @ v_h   per 64-dim head, reassembled

Sharding: batch-parallel, one batch element per NeuronCore (8 cores).
Host-side prep (numpy, layout only): transpose weights to [di, do] and build
ctxT = concat(hidden, context).transpose -> [D, NK] per batch so the
contraction dim lands on SBUF partitions.

Structure: V projection first, then one fused loop over head pairs that
computes the K/Q projection slices for that pair and immediately runs
attention on them.  Scores for the head pair run as concurrent row-tiled
matmuls at partition offsets 0/64 (measured ~3x on HW vs sequential).
Softmax denominators come for free from a ones-column appended to v.
Scores/projections run in float32r (fp32 at FP22, full PE rate); probs and
v are fp16 (memory + out-matmul; ~5e-4 relative error contribution).

Biases are all-zero for this problem spec and are ignored.
"""
import numpy as np

import concourse.bass as bass
import concourse.mybir as mybir
import concourse.tile as tile
from concourse import bacc
from concourse.bass import ds, ts
from concourse.bass_utils import run_bass_kernel_spmd
from concourse.masks import make_identity

N_CORES = 8
P = 128
D = 1024          # model dim
NQ = 1024         # query length (hidden)
NK = 1280         # key/value length (hidden + context)
H = 16            # heads
DH = 64           # head dim
DT = D // P       # 8 contraction tiles
NKT = NK // P     # 10 nk tiles
SCALE = 1.0 / 8.0  # 1/sqrt(DH)
F32 = mybir.dt.float32
F32R = mybir.dt.float32r
F16 = mybir.dt.float16
NQC = 512         # nq chunk for attention
NCH = NQ // NQC   # 2 chunks


def emit(nc, tc, ctx_d, wq_d, wk_d, wv_d, out_d, repeat=1):
    with (
        tc.tile_pool(name="persist", bufs=1) as persist,
        tc.tile_pool(name="wp", bufs=16) as wp,
        tc.tile_pool(name="kqp", bufs=2) as kqp,
        tc.tile_pool(name="p2", bufs=4) as p2,
        tc.tile_pool(name="stg", bufs=4) as stg,
        tc.tile_pool(name="psp", bufs=2, space="PSUM") as psp,
        tc.tile_pool(name="pss", bufs=2, space="PSUM") as pss,
        tc.tile_pool(name="pso", bufs=1, space="PSUM") as pso,
        tc.tile_pool(name="pst", bufs=1, space="PSUM") as pst,
    ):
        ident = persist.tile([P, P], F32)
        make_identity(nc, ident[:])
        ones_d = nc.inline_tensor(np.ones((P, NKT * H), dtype=np.float16),
                                  name="ones")
        for _ in range(repeat):
            _emit_iter(nc, tc, persist, wp, kqp, p2, stg, psp, pss, pso, pst,
                       ident, ones_d, ctx_d, wq_d, wk_d, wv_d, out_d)


def _emit_iter(nc, tc, persist, wp, kqp, p2, stg, psp, pss, pso, pst,
               ident, ones_d, ctx_d, wq_d, wk_d, wv_d, out_d):
    v = persist.tile([P, NKT, H, DH + 1], F16, tag="v")  # natural v + ones col
    nc.sync.dma_start(
        v[:, :, :, DH:DH + 1],
        ones_d[:, :].rearrange("p (t h) -> p t h", t=NKT)[:, :, :, None],
    )

    ctxT = persist.tile([P, DT, NK], F32R, tag="ctxT")

    def load_w(w_d, name):
        tiles = []
        for t in range(DT):
            wt = wp.tile([P, D], F32R, tag="w", name=f"{name}_{t}")
            nc.sync.dma_start(wt[:], w_d[ts(t, P), :].bitcast(F32R))
            tiles.append(wt)
        return tiles

    # DMA order: ctxT and wv feed the V projection that runs first.
    for t in range(DT):
        nc.sync.dma_start(ctxT[:, t, :], ctx_d[ts(t, P), :].bitcast(F32R))
    wv = load_w(wv_d, "wv")
    wk = load_w(wk_d, "wk")
    wq = load_w(wq_d, "wq")

    # ---- V projection: v[nk, do] = sum_di ctxT[di, nk] * WvT[di, do] ----
    for m in range(NKT):
        for g in range(2):
            ps = psp.tile([P, 512], F32, tag="ps")
            for di in range(DT):
                nc.tensor.matmul(
                    ps[:],
                    ctxT[:, di, ts(m, P)],
                    wv[di][:, ds(g * 512, 512)],
                    start=(di == 0),
                    stop=(di == DT - 1),
                )
            nc.vector.tensor_copy(
                v[:, m, ds(g * 8, 8), 0:DH],
                ps[:].rearrange("p (h d) -> p h d", h=8),
            )

    # ---- fused loop over head pairs ----
    for hp in range(H // 2):
        pair = (2 * hp, 2 * hp + 1)
        # K slice for this pair: kT[do=hp-tile, nk]
        kT = kqp.tile([P, NK], F32R, tag="kT", name=f"kT_{hp}")
        for (c0, w) in ((0, 512), (512, 512), (1024, 256)):
            ps = psp.tile([P, 512], F32, tag="ps")
            for di in range(DT):
                nc.tensor.matmul(
                    ps[:, :w],
                    wk[di][:, ts(hp, P)],
                    ctxT[:, di, ds(c0, w)],
                    start=(di == 0),
                    stop=(di == DT - 1),
                )
            nc.vector.tensor_copy(kT[:, ds(c0, w)], ps[:, :w])
        # Q slice for this pair: qT[do=hp-tile, nq]
        qT = kqp.tile([P, NQ], F32R, tag="qT", name=f"qT_{hp}")
        for c in range(2):
            ps = psp.tile([P, 512], F32, tag="ps")
            for di in range(DT):
                nc.tensor.matmul(
                    ps[:],
                    wq[di][:, ts(hp, P)],
                    ctxT[:, di, ds(c * 512, 512)],
                    start=(di == 0),
                    stop=(di == DT - 1),
                )
            nc.vector.tensor_copy(qT[:, ds(c * 512, 512)], ps[:])

        for c in range(NCH):
            et = {
                h: p2.tile([P, NKT, NQC], F16, tag="expT", name=f"expT_{h}")
                for h in pair
            }
            # scoresT[nk, nq]: head pair at partition offsets 0/64 emitted
            # interleaved -> concurrent row-tiled matmuls; 2 nk-tiles share
            # a 2-bank psum tile so exp runs as one big ACT instruction.
            for g in range(NKT // 2):
                pp = {
                    h: pss.tile([P, 2, NQC], F32, tag="pss", name=f"pss_{h}")
                    for h in pair
                }
                for tt in range(2):
                    for h in pair:
                        o = 64 * (h % 2)
                        nc.tensor.matmul(
                            pp[h][:, tt, :],
                            kT[o:o + DH, ts(2 * g + tt, P)],
                            qT[o:o + DH, ds(c * NQC, NQC)],
                            start=True,
                            stop=True,
                        )
                for h in pair:
                    nc.scalar.activation(
                        et[h][:, ds(2 * g, 2), :], pp[h][:, :, :],
                        mybir.ActivationFunctionType.Exp,
                        scale=SCALE,
                    )
            # outT_aug[65, nq] = sum_nk v_aug[nk, 65] * expT[nk, nq]
            for h in pair:
                po = pso.tile([DH + 1, NQC], F32, tag="pso")
                for t in range(NKT):
                    nc.tensor.matmul(
                        po[:],
                        v[:, t, h, :],
                        et[h][:, t, :],
                        start=(t == 0),
                        stop=(t == NKT - 1),
                    )
                st = stg.tile([DH + 1, NQC], F32, tag="stage")
                nc.vector.tensor_copy(st[:], po[:])
                for j in range(NQC // P):
                    pt = pst.tile([P, DH + 1], F32, tag="pst")
                    nc.tensor.transpose(
                        pt[:], st[:, ts(j, P)], ident[:DH + 1, :DH + 1]
                    )
                    rc = stg.tile([P, 1], F32, tag="recip")
                    nc.vector.reciprocal(rc[:], pt[:, DH:DH + 1])
                    ot = stg.tile([P, DH], F32, tag="outstg")
                    nc.vector.tensor_scalar_mul(ot[:], pt[:, 0:DH], rc[:])
                    nt = c * (NQC // P) + j
                    nc.sync.dma_start(
                        out_d[ts(nt, P), ds(h * DH, DH)], ot[:]
                    )


_CACHE = {}


def build(repeat=1):
    key = repeat
    if key in _CACHE:
        return _CACHE[key]
    nc = bacc.Bacc("TRN2", target_bir_lowering=False, debug=False,
                   num_devices=N_CORES)
    ctx_d = nc.dram_tensor("ctxT", [D, NK], F32, kind="ExternalInput")
    wq_d = nc.dram_tensor("wqT", [D, D], F32, kind="ExternalInput")
    wk_d = nc.dram_tensor("wkT", [D, D], F32, kind="ExternalInput")
    wv_d = nc.dram_tensor("wvT", [D, D], F32, kind="ExternalInput")
    out_d = nc.dram_tensor("out", [NQ, D], F32, kind="ExternalOutput")
    with tile.TileContext(nc) as tc:
        emit(nc, tc, ctx_d, wq_d, wk_d, wv_d, out_d, repeat=repeat)
    nc.compile()
    _CACHE[key] = (nc, ctx_d, wq_d, wk_d, wv_d, out_d)
    return _CACHE[key]


def make_in_maps(hidden_states, context_states, Wq, Wk, Wv):
    ctxT = np.ascontiguousarray(
        np.concatenate([hidden_states, context_states], axis=1).transpose(0, 2, 1)
    ).astype(np.float32)
    wqT = np.ascontiguousarray(np.asarray(Wq, dtype=np.float32).T)
    wkT = np.ascontiguousarray(np.asarray(Wk, dtype=np.float32).T)
    wvT = np.ascontiguousarray(np.asarray(Wv, dtype=np.float32).T)
    return [
        {"ctxT": ctxT[b], "wqT": wqT, "wkT": wkT, "wvT": wvT}
        for b in range(N_CORES)
    ]


def kernel(hidden_states, context_states, Wq, bq, Wk, bk, Wv, bv):
    # bq/bk/bv are zeros per the problem spec; not applied.
    nc = build(repeat=1)[0]
    in_maps = make_in_maps(hidden_states, context_states, Wq, Wk, Wv)
    res = run_bass_kernel_spmd(nc, in_maps, core_ids=list(range(N_CORES)))
    return np.stack([res.results[b]["out"] for b in range(N_CORES)], axis=0)


# revision 17
# speedup vs baseline: 6.5324x; 1.4703x over previous
"""Trainium2 Bass kernel for BaseViTSelfAttention (cross/self attention, 16 heads).

Computation (per batch element b):
    q = hidden @ Wq.T            [1024, 1024]
    ctx = concat(hidden, context)  [1280, 1024]
    k = ctx @ Wk.T; v = ctx @ Wv.T
    out = softmax(q_h @ k_h.T / 8) @ v_h   per 64-dim head, reassembled

Sharding: batch-parallel, one batch element per NeuronCore (8 cores).
Host-side prep (numpy, layout only): transpose weights to [di, do] and build
ctxT = concat(hidden, context).transpose -> [D, NK] per batch so the
contraction dim lands on SBUF partitions.

Structure: V projection first, then one fused loop over head pairs that
computes the K/Q projection slices for that pair and immediately runs
attention on them.  Scores for the head pair run as concurrent row-tiled
matmuls at partition offsets 0/64 (measured ~3x on HW vs sequential).
Softmax denominators come for free from a ones-column appended to v.
All matmuls run in fp16 with fp32 PSUM accumulation (separate LDWEIGHTS
hides the weight-load; fp32r self-loading matmuls pay ~25% extra).  The
softmax operates on scores/8 ~ N(0,1), so fp16 rounding of q/k/probs/v
contributes only ~1e-3 relative error overall.

Biases are all-zero for this problem spec and are ignored.
"""
import numpy as np

import concourse.bass as bass
import concourse.mybir as mybir
import concourse.tile as tile
from concourse import bacc
from concourse.bass import ds, ts
from concourse.bass_utils import run_bass_kernel_spmd
from concourse.masks import make_identity

N_CORES = 8
P = 128
D = 1024          # model dim
NQ = 1024         # query length (hidden)
NK = 1280         # key/value length (hidden + context)
H = 16            # heads
DH = 64           # head dim
DT = D // P       # 8 contraction tiles
NKT = NK // P     # 10 nk tiles
SCALE = 1.0 / 8.0  # 1/sqrt(DH)
F32 = mybir.dt.float32
F32R = mybir.dt.float32r
F16 = mybir.dt.float16
NQC = 512         # nq chunk for attention
NCH = NQ // NQC   # 2 chunks


def emit(nc, tc, ctx_d, wq_d, wk_d, wv_d, out_d, repeat=1):
    with (
        tc.tile_pool(name="persist", bufs=1) as persist,
        tc.tile_pool(name="wp", bufs=16) as wp,
        tc.tile_pool(name="kqp", bufs=2) as kqp,
        tc.tile_pool(name="p2", bufs=4) as p2,
        tc.tile_pool(name="stg", bufs=4) as stg,
        tc.tile_pool(name="psp", bufs=2, space="PSUM") as psp,
        tc.tile_pool(name="pss", bufs=2, space="PSUM") as pss,
        tc.tile_pool(name="pso", bufs=1, space="PSUM") as pso,
        tc.tile_pool(name="pst", bufs=1, space="PSUM") as pst,
    ):
        ident = persist.tile([P, P], F32)
        make_identity(nc, ident[:])
        ones_d = nc.inline_tensor(np.ones((P, NKT * H), dtype=np.float16),
                                  name="ones")
        if repeat == 1:
            _emit_iter(nc, tc, persist, wp, kqp, p2, stg, psp, pss, pso, pst,
                       ident, ones_d, ctx_d, wq_d, wk_d, wv_d, out_d)
        else:
            # hardware loop: used only for wall-clock timing builds
            with tc.For_i(0, repeat, 1):
                _emit_iter(nc, tc, persist, wp, kqp, p2, stg, psp, pss, pso,
                           pst, ident, ones_d, ctx_d, wq_d, wk_d, wv_d, out_d)


def _emit_iter(nc, tc, persist, wp, kqp, p2, stg, psp, pss, pso, pst,
               ident, ones_d, ctx_d, wq_d, wk_d, wv_d, out_d):
    v = persist.tile([P, NKT, H, DH + 1], F16, tag="v")  # natural v + ones col
    nc.vector.memset(v[:, :, :, DH:DH + 1], 1.0)

    ctxT = persist.tile([P, DT, NK], F16, tag="ctxT")

    def load_w(w_d, name, eng):
        tiles = []
        for t in range(DT):
            wt = wp.tile([P, D], F16, tag="w", name=f"{name}_{t}")
            eng.dma_start(wt[:], w_d[ts(t, P), :])
            tiles.append(wt)
        return tiles

    # DMA order: ctxT and wv (interleaved, g=0 halves first) feed the V
    # projection that runs first; the first V groups start on half the di
    # range so the PE ramps with the DMA inflow.
    wv = []
    for t in range(DT):
        nc.sync.dma_start(ctxT[:, t, :], ctx_d[ts(t, P), :])
        wt = wp.tile([P, D], F16, tag="w", name=f"wv_{t}")
        nc.sync.dma_start(wt[:, 0:512], wv_d[ts(t, P), 0:512])
        wv.append(wt)
    for t in range(DT):
        nc.sync.dma_start(wv[t][:, 512:1024], wv_d[ts(t, P), 512:1024])
    wk = load_w(wk_d, "wk", nc.sync)
    wq = load_w(wq_d, "wq", nc.sync)

    # ---- V projection: v[nk, do] = sum_di ctxT[di, nk] * WvT[di, do] ----
    for m in range(NKT):
        for g in range(2):
            ps = psp.tile([P, 512], F32, tag="ps")
            for di in range(DT):
                nc.tensor.matmul(
                    ps[:],
                    ctxT[:, di, ts(m, P)],
                    wv[di][:, ds(g * 512, 512)],
                    start=(di == 0),
                    stop=(di == DT - 1),
                )
            nc.vector.tensor_copy(
                v[:, m, ds(g * 8, 8), 0:DH],
                ps[:].rearrange("p (h d) -> p h d", h=8),
            )

    # ---- fused loop over head pairs ----
    for hp in range(H // 2):
        pair = (2 * hp, 2 * hp + 1)
        # K slice for this pair: kT[do=hp-tile, nk]
        kT = kqp.tile([P, NK], F16, tag="kT", name=f"kT_{hp}")
        for (c0, w) in ((0, 512), (512, 512), (1024, 256)):
            ps = psp.tile([P, 512], F32, tag="ps")
            for di in range(DT):
                nc.tensor.matmul(
                    ps[:, :w],
                    wk[di][:, ts(hp, P)],
                    ctxT[:, di, ds(c0, w)],
                    start=(di == 0),
                    stop=(di == DT - 1),
                )
            nc.vector.tensor_copy(kT[:, ds(c0, w)], ps[:, :w])
        # Q slice for this pair: qT[do=hp-tile, nq]
        qT = kqp.tile([P, NQ], F16, tag="qT", name=f"qT_{hp}")
        for c in range(2):
            ps = psp.tile([P, 512], F32, tag="ps")
            for di in range(DT):
                nc.tensor.matmul(
                    ps[:],
                    wq[di][:, ts(hp, P)],
                    ctxT[:, di, ds(c * 512, 512)],
                    start=(di == 0),
                    stop=(di == DT - 1),
                )
            nc.vector.tensor_copy(qT[:, ds(c * 512, 512)], ps[:])

        for c in range(NCH):
            otp = [
                stg.tile([P, 2, DH], F32, tag="outstg", name=f"otp_{c}_{j}")
                for j in range(NQC // P)
            ]
            et = {
                h: p2.tile([P, NKT, NQC], F16, tag="expT", name=f"expT_{h}")
                for h in pair
            }
            # scoresT[nk, nq]: head pair at partition offsets 0/64 emitted
            # interleaved -> concurrent row-tiled matmuls; 2 nk-tiles share
            # a 2-bank psum tile so exp runs as one big ACT instruction.
            for g in range(NKT // 2):
                pp = {
                    h: pss.tile([P, 2, NQC], F32, tag="pss", name=f"pss_{h}")
                    for h in pair
                }
                for tt in range(2):
                    for h in pair:
                        o = 64 * (h % 2)
                        nc.tensor.matmul(
                            pp[h][:, tt, :],
                            kT[o:o + DH, ts(2 * g + tt, P)],
                            qT[o:o + DH, ds(c * NQC, NQC)],
                            start=True,
                            stop=True,
                        )
                for h in pair:
                    nc.scalar.activation(
                        et[h][:, ds(2 * g, 2), :], pp[h][:, :, :],
                        mybir.ActivationFunctionType.Exp,
                        scale=SCALE,
                    )
            # outT_aug[65, nq] = sum_nk v_aug[nk, 65] * expT[nk, nq]
            for h in pair:
                po = pso.tile([DH + 1, NQC], F32, tag="pso")
                for t in range(NKT):
                    nc.tensor.matmul(
                        po[:],
                        v[:, t, h, :],
                        et[h][:, t, :],
                        start=(t == 0),
                        stop=(t == NKT - 1),
                    )
                st = stg.tile([DH + 1, NQC], F32, tag="stage")
                nc.vector.tensor_copy(st[:], po[:])
                for j in range(NQC // P):
                    pt = pst.tile([P, DH + 1], F32, tag="pst")
                    nc.tensor.transpose(
                        pt[:], st[:, ts(j, P)], ident[:DH + 1, :DH + 1]
                    )
                    rc = stg.tile([P, 1], F32, tag="recip")
                    nc.vector.reciprocal(rc[:], pt[:, DH:DH + 1])
                    nc.vector.tensor_scalar_mul(
                        otp[j][:, h % 2, :], pt[:, 0:DH], rc[:]
                    )
            for j in range(NQC // P):
                nt = c * (NQC // P) + j
                eng = nc.gpsimd if j % 2 else nc.sync
                # out_d layout [H/2, NQ, 2*DH]: one contiguous 64KB block
                eng.dma_start(out_d[hp, ts(nt, P), :], otp[j][:])


_CACHE = {}


def build(repeat=1):
    key = repeat
    if key in _CACHE:
        return _CACHE[key]
    nc = bacc.Bacc("TRN2", target_bir_lowering=False, debug=False,
                   num_devices=N_CORES)
    ctx_d = nc.dram_tensor("ctxT", [D, NK], F16, kind="ExternalInput")
    wq_d = nc.dram_tensor("wqT", [D, D], F16, kind="ExternalInput")
    wk_d = nc.dram_tensor("wkT", [D, D], F16, kind="ExternalInput")
    wv_d = nc.dram_tensor("wvT", [D, D], F16, kind="ExternalInput")
    out_d = nc.dram_tensor("out", [H // 2, NQ, 2 * DH], F32,
                           kind="ExternalOutput")
    with tile.TileContext(nc) as tc:
        emit(nc, tc, ctx_d, wq_d, wk_d, wv_d, out_d, repeat=repeat)
    nc.compile()
    _CACHE[key] = (nc, ctx_d, wq_d, wk_d, wv_d, out_d)
    return _CACHE[key]


def make_in_maps(hidden_states, context_states, Wq, Wk, Wv):
    ctxT = np.ascontiguousarray(
        np.concatenate([hidden_states, context_states], axis=1).transpose(0, 2, 1)
    ).astype(np.float16)
    wqT = np.ascontiguousarray(np.asarray(Wq).T).astype(np.float16)
    wkT = np.ascontiguousarray(np.asarray(Wk).T).astype(np.float16)
    wvT = np.ascontiguousarray(np.asarray(Wv).T).astype(np.float16)
    return [
        {"ctxT": ctxT[b], "wqT": wqT, "wkT": wkT, "wvT": wvT}
        for b in range(N_CORES)
    ]


def kernel(hidden_states, context_states, Wq, bq, Wk, bk, Wv, bv):
    # bq/bk/bv are zeros per the problem spec; not applied.
    nc = build(repeat=1)[0]
    in_maps = make_in_maps(hidden_states, context_states, Wq, Wk, Wv)
    res = run_bass_kernel_spmd(nc, in_maps, core_ids=list(range(N_CORES)))
    # device writes [H/2, NQ, 2*DH]; un-permute to [NQ, D] on host
    return np.stack(
        [
            res.results[b]["out"].transpose(1, 0, 2).reshape(NQ, D)
            for b in range(N_CORES)
        ],
        axis=0,
    )
